# revision 31
# baseline (speedup 1.0000x reference)
"""Erwin transformer block (ball MSA + SwiGLU MLP) on 8 Trainium2 cores.

Data-parallel over balls: core i gets token rows [i*16384, (i+1)*16384)
(= 128 balls of 128 tokens). Weights replicated, no collectives.

Per-core dataflow (token-major fp32 residual stream, bf16 matmul operands):
  norm1 (ACT square+accum, DVE recip+scale; norm weight folded into W)
  yT via PE transpose -> qT,kT feature-major (weight-stationary matmuls;
  pos-encoding + bias folded in as K=4 augmented contraction rows)
  v token-major (yT-stationary matmuls, same aug trick)
  dsq via K=5 matmul on [-2p;|p|^2;1] x [p;1;|p|^2] packs, d = ACT sqrt
  scoresT[k,m] = kT_h^T qT_h + sigma_h*d[k,m]  (bias injected by a sigma*I
  matmul into the same PSUM tile; d is symmetric), attnT_raw = batched ACT
  Exp (safe without max-subtraction: |scores| <= ~10), denominators via PE
  ones-matmul over the key partitions, softmax normalization folded into av
  evacuation through selector-broadcast tiles B = sel^T @ (1/denom).
  av (v-stationary, heads packed into PSUM partition quadrants) -> proj
  (av-stationary => token-major out) + residual -> norm2 -> hT -> w1/w2
  (weight-stationary) -> silu(w1+b1)*(w2+b2) -> w3 (g-stationary =>
  token-major out) + residual -> DMA out.
"""

import sys

sys.path.insert(0, "/opt/trn_rl_repo")

from contextlib import ExitStack

import numpy as np
import ml_dtypes

import concourse.bass as bass
import concourse.mybir as mybir
import concourse.tile as tile
from concourse import bacc
from concourse.bass_utils import run_bass_kernel_spmd

DIM = 256
HEADS = 8
BALL = 128
HEAD_DIM = 32
HIDDEN = 1024
EPS = 1e-6
N_CORES = 8
T = 131072 // N_CORES          # tokens per core = 16384
NBALLS = T // BALL             # 128
G = 8                          # balls per chunk
CT = G * BALL                  # tokens per chunk = 1024
F32 = mybir.dt.float32
BF16 = mybir.dt.bfloat16
AF = mybir.ActivationFunctionType
ALU = mybir.AluOpType


def prep_consts(inp):
    """Host-side weight folding. Returns dict name -> np.ndarray."""
    f32 = np.float32
    bf = ml_dtypes.bfloat16
    qkv_w = np.asarray(inp["qkv_w"], f32)        # (768,256), rows h*96+e*3+c
    qkv_b = np.asarray(inp["qkv_b"], f32)
    pe_w = np.asarray(inp["pe_w"], f32)          # (256,3)
    pe_b = np.asarray(inp["pe_b"], f32)
    n1 = np.asarray(inp["norm1_w"], f32)
    n2 = np.asarray(inp["norm2_w"], f32)
    sig = np.asarray(inp["sigma_att"], f32).reshape(HEADS)

    b_eff = qkv_b + qkv_w @ pe_b                 # pe_b folded through qkv_w
    pe_f = qkv_w @ pe_w                          # (768,3) rel-coeffs per output

    hh = np.arange(HEADS)[:, None]
    ee = np.arange(HEAD_DIM)[None, :]
    pq = (hh * 96 + ee * 3 + 0).reshape(-1)      # -> h*32+e row order
    pk = (hh * 96 + ee * 3 + 1).reshape(-1)
    pv = (hh * 96 + ee * 3 + 2).reshape(-1)
    s = 1.0 / np.sqrt(HEAD_DIM)

    c = {}
    c["wqT"] = np.ascontiguousarray((qkv_w[pq] * n1[None, :] * s).T).astype(bf)
    c["wkT"] = np.ascontiguousarray((qkv_w[pk] * n1[None, :]).T).astype(bf)
    wv = (qkv_w[pv] * n1[None, :]).T
    wv_aug = np.zeros((256, 264), f32)
    for h in range(8):
        wv_aug[:, h * 33:h * 33 + 32] = wv[:, h * 32:(h + 1) * 32]
    c["wvT"] = np.ascontiguousarray(wv_aug).astype(bf)
    c["augq"] = np.concatenate([pe_f[pq].T * s, (b_eff[pq] * s)[None, :]]).astype(bf)
    c["augk"] = np.concatenate([pe_f[pk].T, b_eff[pk][None, :]]).astype(bf)
    av4 = np.zeros((4, 264), f32)
    for h in range(8):
        av4[0:3, h * 33:h * 33 + 32] = pe_f[pv].T[:, h * 32:(h + 1) * 32]
        av4[3, h * 33:h * 33 + 32] = b_eff[pv][h * 32:(h + 1) * 32]
        av4[3, h * 33 + 32] = 1.0              # ones column -> denominator
    c["augv"] = av4.astype(bf)
    # proj input rows permuted to the av64 layout: feature (h,e) lives at
    # slot=h//2, partition 32*(h%2)+e  ->  row slot*64 + 32*(h%2) + e
    pT = np.asarray(inp["proj_w"], f32).T
    projT = np.zeros((4, 128, 256), f32)
    for h in range(8):
        for e in range(32):
            projT[h // 2, 64 * (h % 2) + e, :] = pT[h * 32 + e, :]
    c["projT"] = projT.reshape(512, 256).astype(bf)
    c["projb"] = np.asarray(inp["proj_b"], f32)[None, :].astype(bf)
    w1 = np.asarray(inp["w1_w"], f32) * n2[None, :]
    w2 = np.asarray(inp["w2_w"], f32) * n2[None, :]
    c["w1T"] = np.ascontiguousarray(w1.T).astype(bf)       # (256,1024)
    c["w2T"] = np.ascontiguousarray(w2.T).astype(bf)
    # b[p, mt] = b_flat[mt*128 + p]
    c["b1"] = np.asarray(inp["w1_b"], f32).reshape(8, 128).T.copy()
    c["b2"] = np.asarray(inp["w2_b"], f32).reshape(8, 128).T.copy()
    c["w3T"] = np.ascontiguousarray(np.asarray(inp["w3_w"], f32).T).astype(bf)
    c["w3b"] = np.asarray(inp["w3_b"], f32)[None, :].astype(bf)
    eye = np.eye(128, dtype=f32)
    c["sigI"] = (sig[:, None, None] * eye[None]).astype(bf)  # (8,128,128)
    # B-tile selector: den row r = parity*4 + slot at densb[bi*8+r]
    sel_c = np.zeros((64, 8, 4, 128), f32)
    for bi in range(8):
        for slot in range(4):
            for p in range(128):
                if p % 64 < 32:
                    sel_c[bi * 8 + (p // 64) * 4 + slot, bi, slot, p] = 1.0
    c["sel_c"] = sel_c.astype(bf)
    c["ident"] = eye.astype(bf)
    return c


def build_kernel(nc, tc, io, nchunk):
    x_d, pos_d, out_d, C = io["x"], io["pos"], io["out"], io["consts"]
    es: ExitStack = io["es"]

    sing = es.enter_context(tc.tile_pool(name="sing", bufs=1))
    xin = es.enter_context(tc.tile_pool(name="xin", bufs=2))
    wk = es.enter_context(tc.tile_pool(name="wk", bufs=1))
    wk2 = es.enter_context(tc.tile_pool(name="wk2", bufs=2))
    # PSUM pools; per-tag slots: 6 (pb) + 2 (pav) = 8 banks
    pps4 = es.enter_context(tc.tile_pool(name="pps4", bufs=6, space="PSUM"))
    ppsa = es.enter_context(tc.tile_pool(name="ppsa", bufs=2, space="PSUM"))

    def pb():     # rotating big psum (1 bank each, 5 slots)
        return pps4.tile([128, 512], F32, tag="pb", name="pb")

    def pb_n(n, dtype=F32):  # other shapes, same slot class
        return pps4.tile([128, n], dtype, tag="pb", name="pbn")

    # ---- constants ----
    def ld(name, shape, perm=None, **kw):
        ap = C[name]
        t = sing.tile(list(shape), ap.dtype, tag=name)
        nc.sync.dma_start(out=t, in_=ap.rearrange(perm, **kw) if perm else ap)
        return t

    wqT = ld("wqT", (128, 2, 256), "(k p) n -> p k n", k=2)
    wkT = ld("wkT", (128, 2, 256), "(k p) n -> p k n", k=2)
    wvT = ld("wvT", (128, 2, 264), "(k p) n -> p k n", k=2)
    augq, augk, augv = ld("augq", (4, 256)), ld("augk", (4, 256)), ld("augv", (4, 264))
    projT = ld("projT", (128, 4, 256), "(s p) n -> p s n", s=4)
    projb = ld("projb", (1, 256))
    w1T = ld("w1T", (128, 2, 1024), "(k p) n -> p k n", k=2)
    w2T = ld("w2T", (128, 2, 1024), "(k p) n -> p k n", k=2)
    b1, b2 = ld("b1", (128, 8)), ld("b2", (128, 8))
    w3T = ld("w3T", (128, 8, 256), "(k p) n -> p k n", k=8)
    w3b = ld("w3b", (1, 256))
    sigI = ld("sigI", (128, 8, 128), "h p n -> p h n")
    sel_c = ld("sel_c", (64, 8, 4, 128), "k b s p -> k b s p")
    ident = ld("ident", (128, 128))

    epst = sing.tile([128, 1], F32, tag="epst", name="epst")
    nc.vector.memset(epst, EPS)
    ones_row = sing.tile([1, CT], BF16, tag="ones_row", name="ones_row")
    nc.vector.memset(ones_row, 1.0)

    ones3 = sing.tile([3, 1], BF16, tag="ones3", name="ones3")
    nc.vector.memset(ones3, 1.0)

    x_tl = x_d.rearrange("(n p) d -> n p d", p=128)
    out_tl = out_d.rearrange("(n p) d -> n p d", p=128)

    def pos_prep(ci):
        """Per-chunk pos-derived tiles: rel4 (4,CT), plhs/prhs (5,CT)."""
        ct0 = ci * CT
        posTc = wk.tile([3, CT], F32, tag="posTc", name="posTc")
        nc.sync.dma_start(
            out=posTc,
            in_=bass.AP(tensor=pos_d.tensor, offset=pos_d.offset + ct0 * 3,
                        ap=[[1, 3], [3, CT]]))
        mTc = wk.tile([3, G], F32, tag="mTc", name="mTc")
        nc.vector.tensor_reduce(
            out=mTc, in_=posTc.rearrange("c (b m) -> c b m", m=128),
            axis=mybir.AxisListType.X, op=ALU.add, negate=True)
        nc.vector.tensor_scalar_mul(mTc, mTc, 1.0 / 128)
        rel4 = wk.tile([4, CT], BF16, tag="rel4", name="rel4")
        nc.vector.memset(rel4, 1.0)        # row 3 stays ones
        mT_b = bass.AP(tensor=mTc.tensor, offset=mTc.offset,
                       ap=[list(mTc.ap[0]), list(mTc.ap[1]), [0, 128]])
        nc.vector.tensor_tensor(
            out=rel4[0:3].rearrange("c (b m) -> c b m", m=128),
            in0=posTc.rearrange("c (b m) -> c b m", m=128),
            in1=mT_b, op=ALU.add)
        pos_bf = wk.tile([3, CT], BF16, tag="pos_bf", name="pos_bf")
        nc.vector.tensor_copy(pos_bf, posTc)
        psq = wk.tile([3, CT], F32, tag="psq", name="psq")
        nc.vector.tensor_tensor(out=psq, in0=pos_bf, in1=pos_bf, op=ALU.mult)
        psq_bf = wk.tile([3, CT], BF16, tag="psq_bf", name="psq_bf")
        nc.vector.tensor_copy(psq_bf, psq)
        plhs = wk.tile([5, CT], BF16, tag="plhs", name="plhs")
        prhs = wk.tile([5, CT], BF16, tag="prhs", name="prhs")
        nc.vector.memset(plhs, 1.0)        # row 4 stays ones
        nc.vector.memset(prhs, 1.0)        # row 3 stays ones
        nc.vector.tensor_scalar_mul(plhs[0:3], pos_bf, -2.0)
        nc.vector.tensor_copy(prhs[0:3], pos_bf)
        sqrow = wk.tile([1, CT], BF16, tag="sqrow", name="sqrow")
        for cc in range(CT // 512):
            sl = slice(cc * 512, (cc + 1) * 512)
            sq_ps = pps4.tile([1, 512], F32, tag="pb", name="sqps")
            nc.tensor.matmul(sq_ps, ones3, psq_bf[:, sl], start=True, stop=True)
            nc.vector.tensor_copy(sqrow[:, sl], sq_ps)
        nc.sync.dma_start(out=plhs[3:4], in_=sqrow)
        nc.sync.dma_start(out=prhs[4:5], in_=sqrow)
        return rel4, plhs, prhs

    def rmsnorm(xc, tag):
        ssq = wk.tile([128, G], F32, tag=f"ssq{tag}", name=f"ssq{tag}")
        xsq = wk.tile([128, 256], F32, tag=f"xsq{tag}", name=f"xsq{tag}")
        for i in range(G):
            nc.scalar.activation(out=xsq, in_=xc[:, i], func=AF.Square,
                                 accum_out=ssq[:, i:i + 1])
        r = wk.tile([128, G], F32, tag=f"r{tag}", name=f"r{tag}")
        nc.scalar.activation(out=r, in_=ssq, func=AF.Sqrt,
                             bias=epst, scale=1.0 / 256)
        nc.vector.reciprocal(out=r, in_=r)
        y = wk.tile([128, G, 256], BF16, tag=f"y{tag}", name=f"y{tag}")
        for i in range(G):
            nc.scalar.activation(out=y[:, i], in_=xc[:, i], func=AF.Copy,
                                 scale=r[:, i:i + 1])
        yT = wk.tile([128, 2, CT], BF16, tag=f"yT{tag}", name=f"yT{tag}", bufs=2)
        for i in range(0, G, 2):
            pp = pb_n(512, BF16)
            for j in range(4):
                bi, hf = i + j // 2, j % 2
                nc.tensor.transpose(pp[:, j * 128:(j + 1) * 128],
                                    y[:, bi, hf * 128:(hf + 1) * 128], ident)
            in_ap = pp.rearrange("p (b h m) -> p b h m", b=2, m=128)
            out_ap = bass.AP(
                tensor=yT.tensor, offset=yT.offset + i * 128,
                ap=[list(yT.ap[0]), [128, 2], [CT, 2], [1, 128]])
            nc.vector.tensor_copy(out_ap, in_ap)
        return yT

    def prep(ci):
        t0p = ci * G
        xcp = xin.tile([128, G, 256], F32, tag="x", name="x")
        for i in range(G):
            nc.sync.dma_start(out=xcp[:, i], in_=x_tl[t0p + i])
        return xcp, rmsnorm(xcp, "1")

    cur = prep(0)
    for ci in range(nchunk):
        t0 = ci * G
        rel_sl, plhs, prhs = pos_prep(ci)
        xc, yT = cur

        # q/k feature-major
        qT = wk.tile([128, 2, CT], BF16, tag="qT", name="qT", bufs=2)
        kT = wk.tile([128, 2, CT], BF16, tag="kT", name="kT", bufs=2)
        for dst, wT, aug in ((qT, wqT, augq), (kT, wkT, augk)):
            for m in range(2):
                for j in range(CT // 512):
                    sl = slice(j * 512, (j + 1) * 512)
                    pp = pb()
                    nc.tensor.matmul(pp, wT[:, 0, m * 128:(m + 1) * 128],
                                     yT[:, 0, sl], start=True, stop=False)
                    nc.tensor.matmul(pp, wT[:, 1, m * 128:(m + 1) * 128],
                                     yT[:, 1, sl], start=False, stop=False)
                    nc.tensor.matmul(pp, aug[:, m * 128:(m + 1) * 128],
                                     rel_sl[:, sl], start=False, stop=True)
                    nc.vector.tensor_copy(dst[:, m, sl], pp)

        # v token-major
        vc = wk.tile([128, G, 264], BF16, tag="v", name="v")
        for i in range(G):
            tsl = slice(i * 128, (i + 1) * 128)
            pv = pb_n(264)
            nc.tensor.matmul(pv, yT[:, 0, tsl], wvT[:, 0], start=True, stop=False)
            nc.tensor.matmul(pv, yT[:, 1, tsl], wvT[:, 1], start=False, stop=False)
            nc.tensor.matmul(pv, rel_sl[:, tsl], augv, start=False, stop=True)
            nc.vector.tensor_copy(vc[:, i], pv)

        # distance tiles (4 balls per PSUM bank), clamp >= 0, sqrt
        dch = wk.tile([128, 2, 512], BF16, tag="d", name="d")
        for hf in range(G // 4):
            pd = pb()
            for j in range(4):
                bsl = slice((hf * 4 + j) * 128, (hf * 4 + j + 1) * 128)
                nc.tensor.matmul(pd[:, j * 128:(j + 1) * 128],
                                 plhs[:, bsl], prhs[:, bsl], start=True, stop=True)
            nc.vector.tensor_scalar_max(pd, pd, 0.0)
            nc.scalar.activation(out=dch[:, hf], in_=pd, func=AF.Sqrt)

        # base-96 head slices are not addressable as matmul operands:
        # copy rows 96:128 of qT/kT to base-0 tiles via SBUF-SBUF DMA
        q3 = wk.tile([32, 2, CT], BF16, tag="q3", name="q3")
        k3 = wk.tile([32, 2, CT], BF16, tag="k3", name="k3")
        for kc in range(2):
            nc.sync.dma_start(out=q3[:, kc], in_=qT[96:128, kc])
            nc.sync.dma_start(out=k3[:, kc], in_=kT[96:128, kc])

        # per (4-ball half, head): sigma*d inject (1 matmul over the 4-ball
        # d block) + 4 per-ball score matmuls + one batched exp; then densum
        attnT = wk.tile([128, 8, CT], BF16, tag="attnT", name="attnT")
        for hf in range(2):
            for h in range(8):
                pss = pb()
                nc.tensor.matmul(pss, sigI[:, h], dch[:, hf],
                                 start=True, stop=False)
                for j in range(4):
                    bi = hf * 4 + j
                    csl = slice(bi * 128, (bi + 1) * 128)
                    fsl = slice(j * 128, (j + 1) * 128)
                    if h % 4 == 3:
                        lhs, rhs = k3[:, h // 4, csl], q3[:, h // 4, csl]
                    else:
                        psl = slice((h % 4) * 32, (h % 4 + 1) * 32)
                        lhs, rhs = kT[psl, h // 4, csl], qT[psl, h // 4, csl]
                    nc.tensor.matmul(pss[:, fsl], lhs, rhs,
                                     start=False, stop=(j == 3))
                nc.scalar.activation(
                    out=attnT[:, h, hf * 512:(hf + 1) * 512],
                    in_=pss, func=AF.Exp)
        if ci + 1 < nchunk:
            cur = prep(ci + 1)

        # av: ones-column in v makes row 32/96 of each slot the softmax
        # denominator. Raw evac + SBUF->SBUF den DMA, batched recip, then
        # one bf16 normalize pass with B = sel_c^T @ rho.
        av_raw = wk.tile([128, 4, CT], BF16, tag="avr", name="avr")
        densb = wk.tile([64, 128], BF16, tag="densb", name="densb")
        for bi in range(G):
            pav = ppsa.tile([128, 4, 128], F32, tag="pav", name="pav")
            for h in range(8):
                nc.tensor.matmul(pav[64 * (h % 2):64 * (h % 2) + 33, h // 2, :],
                                 vc[:, bi, h * 33:(h + 1) * 33],
                                 attnT[:, h, bi * 128:(bi + 1) * 128],
                                 start=True, stop=True)
            asl = av_raw[:, :, bi * 128:(bi + 1) * 128]
            nc.vector.tensor_copy(asl[0:33], pav[0:33])
            nc.vector.tensor_copy(asl[64:97], pav[64:97])
            nc.sync.dma_start(out=densb[bi * 8:bi * 8 + 4, :], in_=asl[32:33])
            nc.sync.dma_start(out=densb[bi * 8 + 4:bi * 8 + 8, :], in_=asl[96:97])
        rho = wk.tile([64, 128], F32, tag="rho", name="rho")
        nc.vector.reciprocal(out=rho, in_=densb)
        rho_bf = wk.tile([64, 128], BF16, tag="rho_bf", name="rho_bf")
        nc.vector.tensor_copy(rho_bf, rho)
        av_sb = wk.tile([128, 4, CT], BF16, tag="av", name="av")
        nc.vector.memset(av_sb[32:64], 0.0)
        nc.vector.memset(av_sb[64:128], 0.0)
        for bi in range(G):
            pB = pps4.tile([128, 4, 128], F32, tag="pb", name="pB64")
            for slot in range(4):
                nc.tensor.matmul(pB[:, slot, :], sel_c[:, bi, slot, :], rho_bf,
                                 start=True, stop=True)
            Bsb = wk2.tile([128, 4, 128], BF16, tag="Bsb", name="Bsb")
            nc.vector.tensor_copy(Bsb, pB)
            nc.vector.tensor_tensor(
                out=av_sb[0:33, :, bi * 128:(bi + 1) * 128],
                in0=av_raw[0:33, :, bi * 128:(bi + 1) * 128],
                in1=Bsb[0:33], op=ALU.mult)
            nc.vector.tensor_tensor(
                out=av_sb[64:97, :, bi * 128:(bi + 1) * 128],
                in0=av_raw[64:97, :, bi * 128:(bi + 1) * 128],
                in1=Bsb[64:97], op=ALU.mult)

        # proj (token-major out, K=64 chunks from av64 layout) + residual
        x2 = xin.tile([128, G, 256], F32, tag="x2", name="x2")
        for i in range(G):
            tsl = slice(i * 128, (i + 1) * 128)
            pp = pb_n(256)
            for slot in range(4):
                nc.tensor.matmul(pp, av_sb[:, slot, tsl], projT[:, slot],
                                 start=(slot == 0), stop=False)
            nc.tensor.matmul(pp, ones_row[:, tsl], projb, start=False, stop=True)
            nc.vector.tensor_tensor(out=x2[:, i], in0=pp, in1=xc[:, i], op=ALU.add)

        # norm2 + MLP
        hT = rmsnorm(x2, "2")
        x3 = xin.tile([128, G, 256], F32, tag="x3", name="x3", bufs=1)
        for j in range(CT // 512):
            sl = slice(j * 512, (j + 1) * 512)
            sil = wk.tile([128, 8, 512], BF16, tag="sil", name="sil")
            gg = wk.tile([128, 8, 512], BF16, tag="gg", name="gg")
            for mt in range(8):
                msl = slice(mt * 128, (mt + 1) * 128)
                p1 = pb()
                nc.tensor.matmul(p1, w1T[:, 0, msl], hT[:, 0, sl],
                                 start=True, stop=False)
                nc.tensor.matmul(p1, w1T[:, 1, msl], hT[:, 1, sl],
                                 start=False, stop=True)
                nc.scalar.activation(out=sil[:, mt], in_=p1, func=AF.Silu,
                                     bias=b1[:, mt:mt + 1])
                p2 = pb()
                nc.tensor.matmul(p2, w2T[:, 0, msl], hT[:, 0, sl],
                                 start=True, stop=False)
                nc.tensor.matmul(p2, w2T[:, 1, msl], hT[:, 1, sl],
                                 start=False, stop=True)
                nc.vector.scalar_tensor_tensor(
                    out=gg[:, mt], in0=p2, scalar=b2[:, mt:mt + 1],
                    in1=sil[:, mt], op0=ALU.add, op1=ALU.mult)
            for jj in range(4):
                i = j * 4 + jj
                p3 = pb_n(256)
                for kc in range(8):
                    nc.tensor.matmul(p3, gg[:, kc, jj * 128:(jj + 1) * 128],
                                     w3T[:, kc], start=(kc == 0), stop=False)
                nc.tensor.matmul(p3, ones_row[:, i * 128:(i + 1) * 128], w3b,
                                 start=False, stop=True)
                nc.vector.tensor_tensor(out=x3[:, i], in0=p3, in1=x2[:, i],
                                        op=ALU.add)
                nc.sync.dma_start(out=out_tl[t0 + i], in_=x3[:, i])


def trace_and_compile(consts, n_tok=T, n_cores=N_CORES):
    nchunk = n_tok // CT
    nc = bacc.Bacc("TRN2", target_bir_lowering=False, debug=False,
                   enable_asserts=False, num_devices=n_cores)
    io = {"consts": {}}
    io["x"] = nc.dram_tensor("x", (n_tok, DIM), F32, kind="ExternalInput").ap()
    io["pos"] = nc.dram_tensor("pos", (n_tok, 3), F32, kind="ExternalInput").ap()
    io["out"] = nc.dram_tensor("out", (n_tok, DIM), F32, kind="ExternalOutput").ap()
    for name, arr in consts.items():
        io["consts"][name] = nc.dram_tensor(
            name, arr.shape, mybir.dt.from_np(arr.dtype), kind="ExternalInput"
        ).ap()
    global T, NBALLS
    T_saved, NB_saved = T, NBALLS
    T, NBALLS = n_tok, n_tok // BALL
    try:
        with tile.TileContext(nc) as tc:
            with ExitStack() as es:
                io["es"] = es
                build_kernel(nc, tc, io, nchunk)
        nc.compile()
    finally:
        T, NBALLS = T_saved, NB_saved
    return nc


def kernel(**inputs):
    import tempfile, time as _time

    consts = prep_consts(inputs)
    nc = trace_and_compile(consts)
    x = np.ascontiguousarray(np.asarray(inputs["x"], np.float32))
    pos = np.ascontiguousarray(np.asarray(inputs["pos"], np.float32))
    in_maps = []
    for c in range(N_CORES):
        m = {"x": x[c * T:(c + 1) * T], "pos": pos[c * T:(c + 1) * T]}
        m.update(consts)
        in_maps.append(m)
    # trace=True is the execution path validated on hardware (the NTFF hook
    # warms PJRT before the execute); retry to ride out transient device
    # wedges, which clear after a terminal-side reset.
    last_err = None
    for attempt in range(4):
        try:
            res = run_bass_kernel_spmd(nc, in_maps, core_ids=list(range(N_CORES)),
                                       trace=True, tmpdir=tempfile.mkdtemp())
            break
        except Exception as e:  # noqa: BLE001
            last_err = e
            _time.sleep(90)
    else:
        raise last_err
    out = np.concatenate([res.results[c]["out"] for c in range(N_CORES)], axis=0)
    return out.astype(np.float32)


# revision 32
# speedup vs baseline: 1.1004x; 1.1004x over previous
"""Erwin transformer block (ball MSA + SwiGLU MLP) on 8 Trainium2 cores.

Data-parallel over balls: core i gets token rows [i*16384, (i+1)*16384)
(= 128 balls of 128 tokens). Weights replicated, no collectives.

Per-core dataflow (token-major fp32 residual stream, bf16 matmul operands):
  norm1 (ACT square+accum, DVE recip+scale; norm weight folded into W)
  yT via PE transpose -> qT,kT feature-major (weight-stationary matmuls;
  pos-encoding + bias folded in as K=4 augmented contraction rows)
  v token-major (yT-stationary matmuls, same aug trick)
  dsq via K=5 matmul on [-2p;|p|^2;1] x [p;1;|p|^2] packs, d = ACT sqrt
  scoresT[k,m] = kT_h^T qT_h + sigma_h*d[k,m]  (bias injected by a sigma*I
  matmul into the same PSUM tile; d is symmetric), attnT_raw = batched ACT
  Exp (safe without max-subtraction: |scores| <= ~10), denominators via PE
  ones-matmul over the key partitions, softmax normalization folded into av
  evacuation through selector-broadcast tiles B = sel^T @ (1/denom).
  av (v-stationary, heads packed into PSUM partition quadrants) -> proj
  (av-stationary => token-major out) + residual -> norm2 -> hT -> w1/w2
  (weight-stationary) -> silu(w1+b1)*(w2+b2) -> w3 (g-stationary =>
  token-major out) + residual -> DMA out.
"""

import sys

sys.path.insert(0, "/opt/trn_rl_repo")

from contextlib import ExitStack

import numpy as np
import ml_dtypes

import concourse.bass as bass
import concourse.mybir as mybir
import concourse.tile as tile
from concourse import bacc
from concourse.bass_utils import run_bass_kernel_spmd

DIM = 256
HEADS = 8
BALL = 128
HEAD_DIM = 32
HIDDEN = 1024
EPS = 1e-6
N_CORES = 8
T = 131072 // N_CORES          # tokens per core = 16384
NBALLS = T // BALL             # 128
G = 8                          # balls per chunk
CT = G * BALL                  # tokens per chunk = 1024
F32 = mybir.dt.float32
BF16 = mybir.dt.bfloat16
AF = mybir.ActivationFunctionType
ALU = mybir.AluOpType


def prep_consts(inp):
    """Host-side weight folding. Returns dict name -> np.ndarray."""
    f32 = np.float32
    bf = ml_dtypes.bfloat16
    qkv_w = np.asarray(inp["qkv_w"], f32)        # (768,256), rows h*96+e*3+c
    qkv_b = np.asarray(inp["qkv_b"], f32)
    pe_w = np.asarray(inp["pe_w"], f32)          # (256,3)
    pe_b = np.asarray(inp["pe_b"], f32)
    n1 = np.asarray(inp["norm1_w"], f32)
    n2 = np.asarray(inp["norm2_w"], f32)
    sig = np.asarray(inp["sigma_att"], f32).reshape(HEADS)

    b_eff = qkv_b + qkv_w @ pe_b                 # pe_b folded through qkv_w
    pe_f = qkv_w @ pe_w                          # (768,3) rel-coeffs per output

    hh = np.arange(HEADS)[:, None]
    ee = np.arange(HEAD_DIM)[None, :]
    pq = (hh * 96 + ee * 3 + 0).reshape(-1)      # -> h*32+e row order
    pk = (hh * 96 + ee * 3 + 1).reshape(-1)
    pv = (hh * 96 + ee * 3 + 2).reshape(-1)
    s = 1.0 / np.sqrt(HEAD_DIM)

    c = {}
    c["wqT"] = np.ascontiguousarray((qkv_w[pq] * n1[None, :] * s).T).astype(bf)
    c["wkT"] = np.ascontiguousarray((qkv_w[pk] * n1[None, :]).T).astype(bf)
    c["wvT"] = np.ascontiguousarray((qkv_w[pv] * n1[None, :]).T).astype(bf)
    c["augq"] = np.concatenate([pe_f[pq].T * s, (b_eff[pq] * s)[None, :]]).astype(bf)
    c["augk"] = np.concatenate([pe_f[pk].T, b_eff[pk][None, :]]).astype(bf)
    c["augv"] = np.concatenate([pe_f[pv].T, b_eff[pv][None, :]]).astype(bf)
    # proj input rows permuted to the av64 layout: feature (h,e) lives at
    # slot=h//2, partition 32*(h%2)+e  ->  row slot*64 + 32*(h%2) + e
    permP = np.empty(256, np.int64)
    for h in range(8):
        for e in range(32):
            permP[(h // 2) * 64 + (h % 2) * 32 + e] = h * 32 + e
    c["projT"] = np.ascontiguousarray(np.asarray(inp["proj_w"], f32).T[permP]).astype(bf)
    c["projb"] = np.asarray(inp["proj_b"], f32)[None, :].astype(bf)
    w1 = np.asarray(inp["w1_w"], f32) * n2[None, :]
    w2 = np.asarray(inp["w2_w"], f32) * n2[None, :]
    c["w1T"] = np.ascontiguousarray(w1.T).astype(bf)       # (256,1024)
    c["w2T"] = np.ascontiguousarray(w2.T).astype(bf)
    # b[p, mt] = b_flat[mt*128 + p]
    c["b1"] = np.asarray(inp["w1_b"], f32).reshape(8, 128).T.copy()
    c["b2"] = np.asarray(inp["w2_b"], f32).reshape(8, 128).T.copy()
    c["w3T"] = np.ascontiguousarray(np.asarray(inp["w3_w"], f32).T).astype(bf)
    c["w3b"] = np.asarray(inp["w3_b"], f32)[None, :].astype(bf)
    eye = np.eye(128, dtype=f32)
    c["sigI"] = (sig[:, None, None] * eye[None]).astype(bf)  # (8,128,128)
    # densum one-hot: lhsT = hot[:, c, :] has ones in column c%32
    hot = np.zeros((128, 32, 32), f32)
    for cc in range(32):
        hot[:, cc, cc] = 1.0
    c["hot"] = hot.astype(bf)
    # B-tile selector: lhsT = sel_c[:, bi, slot, :] (64 K-rows, 64 M-cols);
    # nonzero at (k = bi*8 + 2*slot + p//32, p)
    sel_c = np.zeros((64, 8, 4, 64), f32)
    for bi in range(8):
        for slot in range(4):
            for p in range(64):
                sel_c[bi * 8 + 2 * slot + p // 32, bi, slot, p] = 1.0
    c["sel_c"] = sel_c.astype(bf)
    c["ident"] = eye.astype(bf)
    return c


def build_kernel(nc, tc, io, nchunk):
    x_d, pos_d, out_d, C = io["x"], io["pos"], io["out"], io["consts"]
    es: ExitStack = io["es"]

    sing = es.enter_context(tc.tile_pool(name="sing", bufs=1))
    xin = es.enter_context(tc.tile_pool(name="xin", bufs=2))
    wk = es.enter_context(tc.tile_pool(name="wk", bufs=1))
    wk2 = es.enter_context(tc.tile_pool(name="wk2", bufs=2))
    # PSUM pools; per-tag slots: 5 (pb) + 2 (pav) + 1 (pden) = 8 banks
    pps4 = es.enter_context(tc.tile_pool(name="pps4", bufs=5, space="PSUM"))
    ppsd = es.enter_context(tc.tile_pool(name="ppsd", bufs=1, space="PSUM"))
    ppsa = es.enter_context(tc.tile_pool(name="ppsa", bufs=2, space="PSUM"))

    def pb():     # rotating big psum (1 bank each, 5 slots)
        return pps4.tile([128, 512], F32, tag="pb", name="pb")

    def pb_n(n, dtype=F32):  # other shapes, same slot class
        return pps4.tile([128, n], dtype, tag="pb", name="pbn")

    # ---- constants ----
    def ld(name, shape, perm=None, **kw):
        ap = C[name]
        t = sing.tile(list(shape), ap.dtype, tag=name)
        nc.sync.dma_start(out=t, in_=ap.rearrange(perm, **kw) if perm else ap)
        return t

    wqT = ld("wqT", (128, 2, 256), "(k p) n -> p k n", k=2)
    wkT = ld("wkT", (128, 2, 256), "(k p) n -> p k n", k=2)
    wvT = ld("wvT", (128, 2, 256), "(k p) n -> p k n", k=2)
    augq, augk, augv = ld("augq", (4, 256)), ld("augk", (4, 256)), ld("augv", (4, 256))
    projT = ld("projT", (64, 4, 256), "(s p) n -> p s n", s=4)
    projb = ld("projb", (1, 256))
    w1T = ld("w1T", (128, 2, 1024), "(k p) n -> p k n", k=2)
    w2T = ld("w2T", (128, 2, 1024), "(k p) n -> p k n", k=2)
    b1, b2 = ld("b1", (128, 8)), ld("b2", (128, 8))
    w3T = ld("w3T", (128, 8, 256), "(k p) n -> p k n", k=8)
    w3b = ld("w3b", (1, 256))
    sigI = ld("sigI", (128, 8, 128), "h p n -> p h n")
    hot = ld("hot", (128, 32, 32))
    sel_c = ld("sel_c", (64, 8, 4, 64), "k b s p -> k b s p")
    ident = ld("ident", (128, 128))

    epst = sing.tile([128, 1], F32, tag="epst", name="epst")
    nc.vector.memset(epst, EPS)
    ones_row = sing.tile([1, CT], BF16, tag="ones_row", name="ones_row")
    nc.vector.memset(ones_row, 1.0)

    ones3 = sing.tile([3, 1], BF16, tag="ones3", name="ones3")
    nc.vector.memset(ones3, 1.0)

    x_tl = x_d.rearrange("(n p) d -> n p d", p=128)
    out_tl = out_d.rearrange("(n p) d -> n p d", p=128)

    def pos_prep(ci):
        """Per-chunk pos-derived tiles: rel4 (4,CT), plhs/prhs (5,CT)."""
        ct0 = ci * CT
        posTc = wk.tile([3, CT], F32, tag="posTc", name="posTc")
        nc.sync.dma_start(
            out=posTc,
            in_=bass.AP(tensor=pos_d.tensor, offset=pos_d.offset + ct0 * 3,
                        ap=[[1, 3], [3, CT]]))
        mTc = wk.tile([3, G], F32, tag="mTc", name="mTc")
        nc.vector.tensor_reduce(
            out=mTc, in_=posTc.rearrange("c (b m) -> c b m", m=128),
            axis=mybir.AxisListType.X, op=ALU.add, negate=True)
        nc.vector.tensor_scalar_mul(mTc, mTc, 1.0 / 128)
        rel4 = wk.tile([4, CT], BF16, tag="rel4", name="rel4")
        nc.vector.memset(rel4, 1.0)        # row 3 stays ones
        mT_b = bass.AP(tensor=mTc.tensor, offset=mTc.offset,
                       ap=[list(mTc.ap[0]), list(mTc.ap[1]), [0, 128]])
        nc.vector.tensor_tensor(
            out=rel4[0:3].rearrange("c (b m) -> c b m", m=128),
            in0=posTc.rearrange("c (b m) -> c b m", m=128),
            in1=mT_b, op=ALU.add)
        pos_bf = wk.tile([3, CT], BF16, tag="pos_bf", name="pos_bf")
        nc.vector.tensor_copy(pos_bf, posTc)
        psq = wk.tile([3, CT], F32, tag="psq", name="psq")
        nc.vector.tensor_tensor(out=psq, in0=pos_bf, in1=pos_bf, op=ALU.mult)
        psq_bf = wk.tile([3, CT], BF16, tag="psq_bf", name="psq_bf")
        nc.vector.tensor_copy(psq_bf, psq)
        plhs = wk.tile([5, CT], BF16, tag="plhs", name="plhs")
        prhs = wk.tile([5, CT], BF16, tag="prhs", name="prhs")
        nc.vector.memset(plhs, 1.0)        # row 4 stays ones
        nc.vector.memset(prhs, 1.0)        # row 3 stays ones
        nc.vector.tensor_scalar_mul(plhs[0:3], pos_bf, -2.0)
        nc.vector.tensor_copy(prhs[0:3], pos_bf)
        sqrow = wk.tile([1, CT], BF16, tag="sqrow", name="sqrow")
        for cc in range(CT // 512):
            sl = slice(cc * 512, (cc + 1) * 512)
            sq_ps = pps4.tile([1, 512], F32, tag="pb", name="sqps")
            nc.tensor.matmul(sq_ps, ones3, psq_bf[:, sl], start=True, stop=True)
            nc.vector.tensor_copy(sqrow[:, sl], sq_ps)
        nc.sync.dma_start(out=plhs[3:4], in_=sqrow)
        nc.sync.dma_start(out=prhs[4:5], in_=sqrow)
        return rel4, plhs, prhs

    def rmsnorm(xc, tag):
        ssq = wk.tile([128, G], F32, tag=f"ssq{tag}", name=f"ssq{tag}")
        xsq = wk.tile([128, 256], F32, tag=f"xsq{tag}", name=f"xsq{tag}")
        for i in range(G):
            nc.scalar.activation(out=xsq, in_=xc[:, i], func=AF.Square,
                                 accum_out=ssq[:, i:i + 1])
        r = wk.tile([128, G], F32, tag=f"r{tag}", name=f"r{tag}")
        nc.scalar.activation(out=r, in_=ssq, func=AF.Sqrt,
                             bias=epst, scale=1.0 / 256)
        nc.vector.reciprocal(out=r, in_=r)
        y = wk.tile([128, G, 256], BF16, tag=f"y{tag}", name=f"y{tag}")
        for i in range(G):
            nc.scalar.activation(out=y[:, i], in_=xc[:, i], func=AF.Copy,
                                 scale=r[:, i:i + 1])
        yT = wk.tile([128, 2, CT], BF16, tag=f"yT{tag}", name=f"yT{tag}", bufs=2)
        for i in range(0, G, 2):
            pp = pb_n(512, BF16)
            for j in range(4):
                bi, hf = i + j // 2, j % 2
                nc.tensor.transpose(pp[:, j * 128:(j + 1) * 128],
                                    y[:, bi, hf * 128:(hf + 1) * 128], ident)
            in_ap = pp.rearrange("p (b h m) -> p b h m", b=2, m=128)
            out_ap = bass.AP(
                tensor=yT.tensor, offset=yT.offset + i * 128,
                ap=[list(yT.ap[0]), [128, 2], [CT, 2], [1, 128]])
            nc.vector.tensor_copy(out_ap, in_ap)
        return yT

    def prep(ci):
        t0p = ci * G
        xcp = xin.tile([128, G, 256], F32, tag="x", name="x")
        for i in range(G):
            nc.sync.dma_start(out=xcp[:, i], in_=x_tl[t0p + i])
        return xcp, rmsnorm(xcp, "1")

    cur = prep(0)
    for ci in range(nchunk):
        t0 = ci * G
        rel_sl, plhs, prhs = pos_prep(ci)
        xc, yT = cur

        # q/k feature-major
        qT = wk.tile([128, 2, CT], BF16, tag="qT", name="qT", bufs=2)
        kT = wk.tile([128, 2, CT], BF16, tag="kT", name="kT", bufs=2)
        for dst, wT, aug in ((qT, wqT, augq), (kT, wkT, augk)):
            for m in range(2):
                for j in range(CT // 512):
                    sl = slice(j * 512, (j + 1) * 512)
                    pp = pb()
                    nc.tensor.matmul(pp, wT[:, 0, m * 128:(m + 1) * 128],
                                     yT[:, 0, sl], start=True, stop=False)
                    nc.tensor.matmul(pp, wT[:, 1, m * 128:(m + 1) * 128],
                                     yT[:, 1, sl], start=False, stop=False)
                    nc.tensor.matmul(pp, aug[:, m * 128:(m + 1) * 128],
                                     rel_sl[:, sl], start=False, stop=True)
                    nc.vector.tensor_copy(dst[:, m, sl], pp)

        # v token-major
        vc = wk.tile([128, G, 256], BF16, tag="v", name="v")
        for i in range(G):
            tsl = slice(i * 128, (i + 1) * 128)
            pv = pb_n(256)
            nc.tensor.matmul(pv, yT[:, 0, tsl], wvT[:, 0], start=True, stop=False)
            nc.tensor.matmul(pv, yT[:, 1, tsl], wvT[:, 1], start=False, stop=False)
            nc.tensor.matmul(pv, rel_sl[:, tsl], augv, start=False, stop=True)
            nc.vector.tensor_copy(vc[:, i], pv)

        # distance tiles (4 balls per PSUM bank), clamp >= 0, sqrt
        dch = wk.tile([128, 2, 512], BF16, tag="d", name="d")
        for hf in range(G // 4):
            pd = pb()
            for j in range(4):
                bsl = slice((hf * 4 + j) * 128, (hf * 4 + j + 1) * 128)
                nc.tensor.matmul(pd[:, j * 128:(j + 1) * 128],
                                 plhs[:, bsl], prhs[:, bsl], start=True, stop=True)
            nc.vector.tensor_scalar_max(pd, pd, 0.0)
            nc.scalar.activation(out=dch[:, hf], in_=pd, func=AF.Sqrt)

        # base-96 head slices are not addressable as matmul operands:
        # copy rows 96:128 of qT/kT to base-0 tiles via SBUF-SBUF DMA
        q3 = wk.tile([32, 2, CT], BF16, tag="q3", name="q3")
        k3 = wk.tile([32, 2, CT], BF16, tag="k3", name="k3")
        for kc in range(2):
            nc.sync.dma_start(out=q3[:, kc], in_=qT[96:128, kc])
            nc.sync.dma_start(out=k3[:, kc], in_=kT[96:128, kc])

        # per (4-ball half, head): sigma*d inject (1 matmul over the 4-ball
        # d block) + 4 per-ball score matmuls + one batched exp; then densum
        attnT = wk.tile([128, 8, CT], BF16, tag="attnT", name="attnT", bufs=2)
        denps = ppsd.tile([64, 128], F32, tag="pden", name="pden")
        for hf in range(2):
            for h in range(8):
                pss = pb()
                nc.tensor.matmul(pss, sigI[:, h], dch[:, hf],
                                 start=True, stop=False)
                for j in range(4):
                    bi = hf * 4 + j
                    csl = slice(bi * 128, (bi + 1) * 128)
                    fsl = slice(j * 128, (j + 1) * 128)
                    if h % 4 == 3:
                        lhs, rhs = k3[:, h // 4, csl], q3[:, h // 4, csl]
                    else:
                        psl = slice((h % 4) * 32, (h % 4 + 1) * 32)
                        lhs, rhs = kT[psl, h // 4, csl], qT[psl, h // 4, csl]
                    nc.tensor.matmul(pss[:, fsl], lhs, rhs,
                                     start=False, stop=(j == 3))
                nc.scalar.activation(
                    out=attnT[:, h, hf * 512:(hf + 1) * 512],
                    in_=pss, func=AF.Exp)
        if ci + 1 < nchunk:
            cur = prep(ci + 1)
        for bi in range(G):
            for h in range(8):
                osl = slice(32 * (bi // 4), 32 * (bi // 4) + 32)
                nc.tensor.matmul(denps[osl, :], hot[:, (bi % 4) * 8 + h, :],
                                 attnT[:, h, bi * 128:(bi + 1) * 128],
                                 start=(bi % 4 == 0 and h == 0),
                                 stop=(bi % 4 == 3 and h == 7))
        rho = wk.tile([64, 128], F32, tag="rho", name="rho")
        nc.vector.reciprocal(out=rho, in_=denps)
        rho_bf = wk.tile([64, 128], BF16, tag="rho_bf", name="rho_bf")
        nc.vector.tensor_copy(rho_bf, rho)

        # av with normalization folded in: B64 = sel_c^T @ rho
        av_sb = wk.tile([64, 4, CT], BF16, tag="av", name="av")
        for bi in range(G):
            pB = pps4.tile([64, 4, 128], F32, tag="pb", name="pB64")
            for slot in range(4):
                nc.tensor.matmul(pB[:, slot, :], sel_c[:, bi, slot, :], rho_bf,
                                 start=True, stop=True)
            Bsb = wk2.tile([64, 4, 128], BF16, tag="Bsb", name="Bsb")
            nc.vector.tensor_copy(Bsb, pB)
            pav = ppsa.tile([64, 4, 128], F32, tag="pav", name="pav")
            for h in range(8):
                nc.tensor.matmul(pav[32 * (h % 2):32 * (h % 2) + 32, h // 2, :],
                                 vc[:, bi, h * 32:(h + 1) * 32],
                                 attnT[:, h, bi * 128:(bi + 1) * 128],
                                 start=True, stop=True)
            nc.vector.tensor_tensor(
                out=av_sb[:, :, bi * 128:(bi + 1) * 128],
                in0=pav, in1=Bsb, op=ALU.mult)

        # proj (token-major out, K=64 chunks from av64 layout) + residual
        x2 = xin.tile([128, G, 256], F32, tag="x2", name="x2")
        for i in range(G):
            tsl = slice(i * 128, (i + 1) * 128)
            pp = pb_n(256)
            for slot in range(4):
                nc.tensor.matmul(pp, av_sb[:, slot, tsl], projT[:, slot],
                                 start=(slot == 0), stop=False)
            nc.tensor.matmul(pp, ones_row[:, tsl], projb, start=False, stop=True)
            nc.vector.tensor_tensor(out=x2[:, i], in0=pp, in1=xc[:, i], op=ALU.add)

        # norm2 + MLP
        hT = rmsnorm(x2, "2")
        x3 = xin.tile([128, G, 256], F32, tag="x3", name="x3", bufs=1)
        for j in range(CT // 512):
            sl = slice(j * 512, (j + 1) * 512)
            sil = wk.tile([128, 8, 512], BF16, tag="sil", name="sil")
            gg = wk.tile([128, 8, 512], BF16, tag="gg", name="gg")
            for mt in range(8):
                msl = slice(mt * 128, (mt + 1) * 128)
                p1 = pb()
                nc.tensor.matmul(p1, w1T[:, 0, msl], hT[:, 0, sl],
                                 start=True, stop=False)
                nc.tensor.matmul(p1, w1T[:, 1, msl], hT[:, 1, sl],
                                 start=False, stop=True)
                nc.scalar.activation(out=sil[:, mt], in_=p1, func=AF.Silu,
                                     bias=b1[:, mt:mt + 1])
                p2 = pb()
                nc.tensor.matmul(p2, w2T[:, 0, msl], hT[:, 0, sl],
                                 start=True, stop=False)
                nc.tensor.matmul(p2, w2T[:, 1, msl], hT[:, 1, sl],
                                 start=False, stop=True)
                nc.vector.scalar_tensor_tensor(
                    out=gg[:, mt], in0=p2, scalar=b2[:, mt:mt + 1],
                    in1=sil[:, mt], op0=ALU.add, op1=ALU.mult)
            for jj in range(4):
                i = j * 4 + jj
                p3 = pb_n(256)
                for kc in range(8):
                    nc.tensor.matmul(p3, gg[:, kc, jj * 128:(jj + 1) * 128],
                                     w3T[:, kc], start=(kc == 0), stop=False)
                nc.tensor.matmul(p3, ones_row[:, i * 128:(i + 1) * 128], w3b,
                                 start=False, stop=True)
                nc.vector.tensor_tensor(out=x3[:, i], in0=p3, in1=x2[:, i],
                                        op=ALU.add)
                nc.sync.dma_start(out=out_tl[t0 + i], in_=x3[:, i])


def trace_and_compile(consts, n_tok=T, n_cores=N_CORES):
    nchunk = n_tok // CT
    nc = bacc.Bacc("TRN2", target_bir_lowering=False, debug=False,
                   enable_asserts=False, num_devices=n_cores)
    io = {"consts": {}}
    io["x"] = nc.dram_tensor("x", (n_tok, DIM), F32, kind="ExternalInput").ap()
    io["pos"] = nc.dram_tensor("pos", (n_tok, 3), F32, kind="ExternalInput").ap()
    io["out"] = nc.dram_tensor("out", (n_tok, DIM), F32, kind="ExternalOutput").ap()
    for name, arr in consts.items():
        io["consts"][name] = nc.dram_tensor(
            name, arr.shape, mybir.dt.from_np(arr.dtype), kind="ExternalInput"
        ).ap()
    global T, NBALLS
    T_saved, NB_saved = T, NBALLS
    T, NBALLS = n_tok, n_tok // BALL
    try:
        with tile.TileContext(nc) as tc:
            with ExitStack() as es:
                io["es"] = es
                build_kernel(nc, tc, io, nchunk)
        nc.compile()
    finally:
        T, NBALLS = T_saved, NB_saved
    return nc


def kernel(**inputs):
    import tempfile, time as _time

    consts = prep_consts(inputs)
    nc = trace_and_compile(consts)
    x = np.ascontiguousarray(np.asarray(inputs["x"], np.float32))
    pos = np.ascontiguousarray(np.asarray(inputs["pos"], np.float32))
    in_maps = []
    for c in range(N_CORES):
        m = {"x": x[c * T:(c + 1) * T], "pos": pos[c * T:(c + 1) * T]}
        m.update(consts)
        in_maps.append(m)
    # trace=True is the execution path validated on hardware (the NTFF hook
    # warms PJRT before the execute); retry to ride out transient device
    # wedges, which clear after a terminal-side reset.
    last_err = None
    for attempt in range(4):
        try:
            res = run_bass_kernel_spmd(nc, in_maps, core_ids=list(range(N_CORES)),
                                       trace=True, tmpdir=tempfile.mkdtemp())
            break
        except Exception as e:  # noqa: BLE001
            last_err = e
            _time.sleep(90)
    else:
        raise last_err
    out = np.concatenate([res.results[c]["out"] for c in range(N_CORES)], axis=0)
    return out.astype(np.float32)
